# revision 22
# baseline (speedup 1.0000x reference)
"""Trainium2 Bass kernel for the data-uncertainty attention module.

Shapes (hardcoded): B=4, N=1024, C=768, H=12, hd=64.
Sharding: 8 cores; core c -> batch b=c//2, query-row half rh=c%2 (512 rows).
Each core computes all 12 heads for its row block, so the 1x1 head-mixing
conv is core-local (du_w replicated). All outputs are disjoint slices;
no collectives.

Math per core (rows n, all heads h, all keys m):
  qkvT = qkv_w @ x^T (+b)  -> qT/kT in [head*64+d, token] layout, v in
                              [token, 768] bf16
  S[h]    = q_h k_h^T            (fp32r matmuls, PSUM)
  expS    = exp(S*scale), rowsum via fused accum  -> bf16, SBUF
  S_all   = raw S -> bf16, SBUF  (for the head-mix)
  mean    = expS * 1/rowsum      -> attn_mean output (f32)
  S_pk    = repack S_all rows to (row-subgroup x head) partitions (SB2SB DMA)
  L       = blockdiag(du_w) @ S_pk   (one 120x120 matmul mixes heads)
  u       = sigmoid(L + du_b)    -> uncertainty output (f32, straight from
                                    the packed tile); bf16 copy back to
                                    row-major (u_all) for the combine
  attn    = mean + u*r = (expS*inv) + u*r   (fused scalar_tensor_tensor,
                                             bf16)
  attnT   = PE transpose (bf16)
  outT    = v^T attnT accumulation -> proj (fp32r) + bias -> out
"""

import numpy as np
import ml_dtypes

import concourse.bass as bass
import concourse.tile as tile
from concourse import bacc, mybir
from concourse.bass_utils import run_bass_kernel_spmd

F32 = mybir.dt.float32
F32R = mybir.dt.float32r
BF16 = mybir.dt.bfloat16
AF = mybir.ActivationFunctionType
ALU = mybir.AluOpType

B, N, C, H = 4, 1024, 768, 12
HD = C // H  # 64
NCORES = 8
NLOC = N // 2          # 512 query rows per core
NRT = NLOC // 128      # 4 row tiles
NG = 13                # 10-row groups per 128-row tile (12x10 + 1x8)
SCALE = HD ** -0.5


def _r(ap):
    return ap.bitcast(F32R)


def build_nc(iters: int = 1):
    nc = bacc.Bacc("TRN2", target_bir_lowering=False, debug=False,
                   num_devices=NCORES)

    # ---- dram parameters (per-core) ----
    xT_d = nc.declare_dram_parameter("xT", [C, N], F32, isOutput=False)
    xqT_d = nc.declare_dram_parameter("xqT", [C, NLOC], F32, isOutput=False)
    r_d = nc.declare_dram_parameter("r", [H, NLOC, N], F32, isOutput=False)
    qkvwT_d = nc.declare_dram_parameter("qkvwT", [C, 3 * C], F32, isOutput=False)
    qkvb_d = nc.declare_dram_parameter("qkvb", [2 * C, 1], F32, isOutput=False)
    projwT_d = nc.declare_dram_parameter("projwT", [C, C], F32, isOutput=False)
    projb_d = nc.declare_dram_parameter("projb_bc", [128, C], F32, isOutput=False)
    vb_d = nc.declare_dram_parameter("vb_bc", [128, C], F32, isOutput=False)
    dubd_d = nc.declare_dram_parameter("du_bd", [120, 120], BF16, isOutput=False)
    dub_d = nc.declare_dram_parameter("du_b_pk", [120, 1], F32, isOutput=False)
    ident_d = nc.declare_dram_parameter("ident", [128, 128], BF16, isOutput=False)

    out_d = nc.declare_dram_parameter("out", [NLOC, C], F32, isOutput=True)
    mean_d = nc.declare_dram_parameter("attn_mean", [H, NLOC, N], BF16, isOutput=True)
    unc_d = nc.declare_dram_parameter("uncertainty", [H, NLOC, N], BF16, isOutput=True)

    with tile.TileContext(nc) as tc:
        with tc.tile_pool(name="wpool", bufs=1) as wpool:
            # persistent tiles
            qT = [wpool.tile([128, NLOC], F32, tag=f"qT{i}", name=f"qT{i}") for i in range(6)]
            kT = [wpool.tile([128, N], F32, tag=f"kT{i}", name=f"kT{i}") for i in range(6)]
            vbf = [wpool.tile([128, C], BF16, tag=f"v{i}", name=f"v{i}") for i in range(8)]
            pwT = [wpool.tile([128, C], F32, tag=f"pw{i}", name=f"pw{i}") for i in range(6)]
            pb_bc = wpool.tile([128, C], F32, tag="pb", name="pb")
            vb_bc = wpool.tile([128, C], F32, tag="vb", name="vb")
            du_bd = wpool.tile([120, 120], BF16, tag="dubd", name="dubd")
            du_b = wpool.tile([120, 1], F32, tag="dub", name="dub")
            ident = wpool.tile([128, 128], BF16, tag="ident", name="ident")
            qb = [wpool.tile([128, 1], F32, tag=f"qb{i}", name=f"qb{i}") for i in range(12)]

            nc.sync.dma_start(out=pb_bc, in_=projb_d.ap())
            nc.sync.dma_start(out=vb_bc, in_=vb_d.ap())
            nc.sync.dma_start(out=du_bd, in_=dubd_d.ap())
            nc.sync.dma_start(out=du_b, in_=dub_d.ap())
            nc.sync.dma_start(out=ident, in_=ident_d.ap())
            for i in range(6):
                nc.sync.dma_start(out=pwT[i], in_=projwT_d[i * 128:(i + 1) * 128, :])
            for i in range(12):
                nc.sync.dma_start(out=qb[i], in_=qkvb_d[i * 128:(i + 1) * 128, :])

            def body():
                # ---------------- phase 1: qkv projections ----------------
                with tc.tile_pool(name="p1", bufs=1) as p1, \
                     tc.tile_pool(name="p1p", bufs=2, space="PSUM") as p1p:
                    wts = [p1.tile([128, 3 * C], F32, tag=f"w{i}", name=f"w{i}") for i in range(6)]
                    xTs = [p1.tile([128, N], F32, tag=f"x{i}", name=f"x{i}") for i in range(6)]
                    xqTs = [p1.tile([128, NLOC], F32, tag=f"xq{i}", name=f"xq{i}") for i in range(6)]
                    for i in range(6):
                        nc.sync.dma_start(out=wts[i],
                                          in_=qkvwT_d[i * 128:(i + 1) * 128, :])
                        nc.sync.dma_start(out=xTs[i],
                                          in_=xT_d[i * 128:(i + 1) * 128, :])
                        nc.sync.dma_start(out=xqTs[i],
                                          in_=xqT_d[i * 128:(i + 1) * 128, :])
                    # q (core's 512 tokens only)
                    for ot in range(6):
                        ps = p1p.tile([128, NLOC], F32, tag="ps_q", name="ps_q")
                        for ct in range(6):
                            nc.tensor.matmul(
                                ps, _r(wts[ct][:, ot * 128:(ot + 1) * 128]),
                                _r(xqTs[ct]), start=(ct == 0), stop=(ct == 5))
                        nc.scalar.activation(qT[ot], ps, AF.Identity, bias=qb[ot])
                    # k (all 1024 tokens)
                    for ot in range(6):
                        for th in range(2):
                            ps = p1p.tile([128, 512], F32, tag="ps_k", name="ps_k")
                            for ct in range(6):
                                nc.tensor.matmul(
                                    ps,
                                    _r(wts[ct][:, C + ot * 128:C + (ot + 1) * 128]),
                                    _r(xTs[ct][:, th * 512:(th + 1) * 512]),
                                    start=(ct == 0), stop=(ct == 5))
                            nc.scalar.activation(
                                kT[ot][:, th * 512:(th + 1) * 512], ps,
                                AF.Identity, bias=qb[6 + ot])
                    # v natural layout [tokens, 768] -> bf16
                    for tt in range(8):
                        ps = p1p.tile([128, C], F32, tag="ps_v", name="ps_v")
                        for (e0, ew) in ((0, 512), (512, 256)):
                            for ct in range(6):
                                nc.tensor.matmul(
                                    ps[:, e0:e0 + ew],
                                    _r(xTs[ct][:, tt * 128:(tt + 1) * 128]),
                                    _r(wts[ct][:, 2 * C + e0:2 * C + e0 + ew]),
                                    start=(ct == 0), stop=(ct == 5))
                        nc.vector.tensor_tensor(vbf[tt], ps, vb_bc, ALU.add)

                # ---------------- phase 2: attention main loop ----------------
                with tc.tile_pool(name="m1", bufs=1) as mp1, \
                     tc.tile_pool(name="m2", bufs=2) as mp2, \
                     tc.tile_pool(name="ps2", bufs=2, space="PSUM") as ps2, \
                     tc.tile_pool(name="ps1", bufs=1, space="PSUM") as ps1:
                    for rt in range(NRT):
                        n0 = rt * 128
                        S_all = mp2.tile([128, H * N], BF16, tag="S_all", name="S_all")
                        u_all = mp1.tile([128, H * N], BF16, tag="u_all", name="u_all")
                        rs = mp2.tile([128, H], F32, tag="rs", name="rs")
                        r_ts = {}
                        negln = mp2.tile([128, H], F32, tag="negln", name="negln")

                        def hsl(t, hh, h):
                            # [128, (head, m)] h-major view of a 12288-wide tile
                            return t[:, h * N + hh * 512:h * N + (hh + 1) * 512]

                        # S = q k^T ; exp/rowsum ; raw-S -> bf16 S_all
                        for h in range(H):
                            ht, hp = h // 2, (h % 2) * 64
                            ps_S = ps2.tile([128, N], F32, tag="ps_S", name="ps_S")
                            for mh in range(2):
                                nc.tensor.matmul(
                                    ps_S[:, mh * 512:(mh + 1) * 512],
                                    _r(qT[ht][hp:hp + 64, n0:n0 + 128]),
                                    _r(kT[ht][hp:hp + 64, mh * 512:(mh + 1) * 512]),
                                    start=True, stop=True)
                            nc.vector.tensor_copy(
                                S_all[:, h * N:(h + 1) * N], ps_S)
                            r_ts[h] = mp2.tile([128, N], F32, tag="r_t",
                                               name="r_t", bufs=4)
                            nc.sync.dma_start(out=r_ts[h],
                                              in_=r_d[h, n0:n0 + 128, :])
                            expt = mp2.tile([128, N], BF16, tag="expt",
                                            name="expt", bufs=1)
                            nc.scalar.activation(
                                expt, S_all[:, h * N:(h + 1) * N],
                                AF.Exp, scale=SCALE,
                                accum_out=rs[:, h:h + 1])
                        lnr = mp2.tile([128, H], F32, tag="lnr", name="lnr")
                        nc.scalar.activation(lnr, rs, AF.Ln)
                        nc.vector.tensor_scalar(negln, lnr, -1.0, None, ALU.mult)
                        # repack S -> (jrow, head) partitions; 120x120
                        # block-diag matmul mixes heads; sigmoid -> u
                        for g in range(NG):
                            gj = 10 if g < 12 else 8
                            gp = gj * H
                            Spk = mp2.tile([120, N], BF16, tag="Spk", name="Spk",
                                           bufs=3)
                            src = S_all[10 * g:10 * g + gj, :].rearrange(
                                "p (h m) -> p h m", h=H)
                            nc.sync.dma_start(out=Spk[0:gp, :], in_=src)
                            L = ps1.tile([120, N], F32, tag="Lproj", name="Lproj")
                            for hh in range(2):
                                nc.tensor.matmul(
                                    L[0:gp, hh * 512:(hh + 1) * 512],
                                    du_bd[0:gp, 0:gp],
                                    Spk[0:gp, hh * 512:(hh + 1) * 512],
                                    start=True, stop=True)
                            upk = mp2.tile([120, N], BF16, tag="upk", name="upk")
                            nc.scalar.activation(upk[0:gp, :], L[0:gp, :],
                                                 AF.Sigmoid, bias=du_b[0:gp, :])
                            # uncertainty out: straight from the packed tile
                            dst = unc_d[:, n0 + 10 * g:n0 + 10 * g + gj, :] \
                                .rearrange("h j m -> j h m")
                            nc.sync.dma_start(out=dst, in_=upk[0:gp, :])
                            # u (bf16) back to row-major for the combine
                            dstu = u_all[10 * g:10 * g + gj, :].rearrange(
                                "p (h m) -> p h m", h=H)
                            nc.sync.dma_start(out=dstu, in_=upk[0:gp, :])
                        # attn = expS*inv + u*r ; transpose ; attn @ v
                        oTs = {}
                        for h in range(H):
                            r_t = r_ts[h]
                            ps_o = ps1.tile([64, 128], F32, tag="ps_o", name="ps_o")
                            mean2 = mp2.tile([128, N], BF16, tag="mean2",
                                             name="mean2", bufs=3)
                            nc.scalar.activation(
                                mean2, S_all[:, h * N:(h + 1) * N],
                                AF.Exp, scale=SCALE, bias=negln[:, h:h + 1])
                            for hh in range(2):
                                ur = mp2.tile([128, 512], F32, tag="ur", name="ur")
                                nc.vector.tensor_tensor(
                                    ur, hsl(u_all, hh, h),
                                    r_t[:, hh * 512:(hh + 1) * 512], ALU.mult)
                                at = mp2.tile([128, 512], BF16, tag="attn", name="attn")
                                nc.vector.tensor_tensor(
                                    at, mean2[:, hh * 512:(hh + 1) * 512], ur,
                                    ALU.add)
                                psT = ps1.tile([128, 512], BF16, tag="psT", name="psT")
                                for mb in range(4):
                                    nc.tensor.transpose(
                                        psT[:, mb * 128:(mb + 1) * 128],
                                        at[:, mb * 128:(mb + 1) * 128], ident)
                                aT = mp2.tile([128, 512], BF16, tag="aT", name="aT")
                                if h % 2 == 0:
                                    nc.vector.tensor_copy(aT, psT)
                                else:
                                    nc.scalar.activation(aT, psT, AF.Copy)
                                for mb in range(4):
                                    mg = hh * 4 + mb
                                    nc.tensor.matmul(
                                        ps_o, vbf[mg][:, h * 64:h * 64 + 64],
                                        aT[:, mb * 128:(mb + 1) * 128],
                                        start=(mg == 0), stop=(mg == 7))
                            if h % 2 == 0:
                                oTs[h // 2] = mp2.tile([128, 128], F32,
                                                       tag=f"oT{h // 2}", name=f"oT{h // 2}")
                            nc.scalar.activation(
                                oTs[h // 2][(h % 2) * 64:(h % 2) * 64 + 64, :],
                                ps_o, AF.Copy)
                        # proj
                        out_f = mp2.tile([128, C], F32, tag="out_f", name="out_f")
                        for (e0, ew) in ((0, 512), (512, 256)):
                            ps_p = ps1.tile([128, 512], F32, tag="Lproj", name="Lproj")
                            for ct in range(6):
                                nc.tensor.matmul(
                                    ps_p[:, 0:ew], _r(oTs[ct]),
                                    _r(pwT[ct][:, e0:e0 + ew]),
                                    start=(ct == 0), stop=(ct == 5))
                            nc.vector.tensor_tensor(out_f[:, e0:e0 + ew],
                                                    ps_p[:, 0:ew],
                                                    pb_bc[:, e0:e0 + ew], ALU.add)
                        nc.sync.dma_start(out=out_d[n0:n0 + 128, :], in_=out_f)

            if iters == 1:
                body()
            elif iters < 0:
                for _ in range(-iters):
                    body()
            else:
                with tc.For_i(0, iters, 1):
                    body()

    nc.compile()
    return nc


def _prep_inputs(x, r, qkv_w, qkv_b, proj_w, proj_b, du_w, du_b):
    x = np.asarray(x, np.float32)
    r = np.asarray(r, np.float32)
    qkv_w = np.asarray(qkv_w, np.float32)
    qkv_b = np.asarray(qkv_b, np.float32)
    proj_w = np.asarray(proj_w, np.float32)
    proj_b = np.asarray(proj_b, np.float32)
    du_w = np.asarray(du_w, np.float32)
    du_b = np.asarray(du_b, np.float32)

    qkvwT = np.ascontiguousarray(qkv_w.T)
    projwT = np.ascontiguousarray(proj_w.T)
    projb_bc = np.ascontiguousarray(np.tile(proj_b[None, :], (128, 1)))
    vb_bc = np.ascontiguousarray(np.tile(qkv_b[None, 2 * C:], (128, 1)))
    qkvb2 = qkv_b[:2 * C].reshape(2 * C, 1).copy()

    # block-diagonal du_w, partitions packed as p = j*12 + h (j = row-in-group)
    du_bd = np.zeros((120, 120), np.float32)
    for j in range(10):
        du_bd[j * H:(j + 1) * H, j * H:(j + 1) * H] = du_w.T  # [h, o] = du_w[o, h]
    du_bd = du_bd.astype(ml_dtypes.bfloat16)
    du_b_pk = np.ascontiguousarray(
        np.tile(du_b[None, :], (10, 1)).reshape(120, 1))
    ident = np.eye(128, dtype=ml_dtypes.bfloat16)

    in_maps = []
    for c in range(NCORES):
        b, rh = c // 2, c % 2
        xT = np.ascontiguousarray(x[b].T)
        in_maps.append({
            "xT": xT,
            "xqT": np.ascontiguousarray(xT[:, rh * NLOC:(rh + 1) * NLOC]),
            "r": np.ascontiguousarray(r[b, :, rh * NLOC:(rh + 1) * NLOC, :]),
            "qkvwT": qkvwT,
            "qkvb": qkvb2,
            "projwT": projwT,
            "projb_bc": projb_bc,
            "vb_bc": vb_bc,
            "du_bd": du_bd,
            "du_b_pk": du_b_pk,
            "ident": ident,
        })
    return in_maps


_cache = {}


def _get_nc(iters=1):
    if iters not in _cache:
        _cache[iters] = build_nc(iters)
    return _cache[iters]


def run_on_cores(in_maps, iters=1):
    nc = _get_nc(iters)
    return run_bass_kernel_spmd(nc, in_maps, list(range(NCORES)))


def kernel(x, r, qkv_w, qkv_b, proj_w, proj_b, du_w, du_b):
    in_maps = _prep_inputs(x, r, qkv_w, qkv_b, proj_w, proj_b, du_w, du_b)
    res = run_on_cores(in_maps).results

    out = np.empty((B, N, C), np.float32)
    attn_mean = np.empty((B, H, N, N), np.float32)
    uncertainty = np.empty((B, H, N, N), np.float32)
    for c in range(NCORES):
        b, rh = c // 2, c % 2
        sl = slice(rh * NLOC, (rh + 1) * NLOC)
        out[b, sl, :] = res[c]["out"]
        attn_mean[b, :, sl, :] = np.asarray(res[c]["attn_mean"],
                                            dtype=np.float32)
        uncertainty[b, :, sl, :] = np.asarray(res[c]["uncertainty"],
                                              dtype=np.float32)
    return out, attn_mean, uncertainty
